# revision 10
# baseline (speedup 1.0000x reference)
"""Trainium2 Bass kernel for nn_MultiHeadAttention_80418967650946.

Reference computation (per batch b):
  qp/kp/vp = 1x1-conv projections of q/k/v   [64, N]
  funky head view: qh[h,n,d] = qp.reshape(4, 16*N)[d, 16n+h]  (same for kh, vh)
  scores = qh @ kh * 0.25^0.5 + bias ; attn = softmax(scores)
  x[4h+d, n] = (attn @ vh)[h, n, d] ; y = LeakyReLU(BN(Wo @ x + bo), 0.2)

Sharding: 8 cores = 4 batches x 2 query-halves (n in [0,512) or [512,1024)).
Each core computes its query-half for ALL 16 heads fully locally (no
collectives): the output conv is column-wise independent, so y[:, n-half]
only needs x[:, n-half].

Per-core device algorithm (all fp32):
  - projections on TensorE produce Kp2 [4, 16384] (d-major, J=16m+h free),
    Qp2 [4, 8192] (pre-scaled by 0.25^0.5), and Vt [128, 1280] where
    Vt[r, 80s+c] = VpT[16r+s, c] for c<64 and 1.0 for c in [64,80)
    (rows 64:128 duplicate 0:64 for the second K=64 row-group).
  - scoresT[m-chunk, n] psum tiles come from one K=4 matmul; the additive
    bias is injected into the SAME psum by an identity matmul over the
    (host-pre-transposed) bias tile; exp() runs on ScalarE psum->sbuf.
  - attn@V contracts m on partitions via two K=64 row-tiled matmuls whose
    lhsT carries a ones column -> softmax denominators come for free.
  - per-head normalization assembles x into a PSUM tile (PSUM APs have no
    32-partition base alignment restriction), then one copy -> SBUF feeds
    the output matmul + fused BN-affine + LeakyReLU epilogue.
"""
import sys

if "/opt/trn_rl_repo" not in sys.path:
    sys.path.insert(0, "/opt/trn_rl_repo")

import numpy as np

import concourse.bass as bass
import concourse.tile as tile
from concourse import bacc, mybir
from concourse.bass_utils import run_bass_kernel_spmd

F32 = mybir.dt.float32
AF = mybir.ActivationFunctionType
ALU = mybir.AluOpType
PSUM = bass.MemorySpace.PSUM

H = 16
D = 4
HID = 256
B = 4
N = 1024
NH = 512          # per-core query positions
NCORES = 8
SCALE = float(D) ** -0.5
BN_EPS = 1e-5
NEG_SLOPE = 0.2


def _emit(nc, tc, io):
    kb, qb, vb = io["kb"], io["qb"], io["vb"]
    biasT, wkT, wvT, wqT, woT = io["biasT"], io["wkT"], io["wvT"], io["wqT"], io["woT"]
    bnv, ident, y = io["bnv"], io["ident"], io["y"]

    with tc.tile_pool(name="persist", bufs=1) as persist:
        _emit_body(nc, tc, io, persist)


def _emit_body(nc, tc, io, persist):
    kb, qb, vb = io["kb"], io["qb"], io["vb"]
    biasT, wkT, wvT, wqT, woT = io["biasT"], io["wkT"], io["wvT"], io["wqT"], io["woT"]
    bnv, ident, y = io["bnv"], io["ident"], io["y"]

    Kp2 = persist.tile([4, H * N], F32, tag="Kp2")
    Qp2 = persist.tile([4, H * NH], F32, tag="Qp2")
    Vt = persist.tile([128, 16 * 80], F32, tag="Vt")
    x_sb = persist.tile([64, NH], F32, tag="x_sb")
    woT_sb = persist.tile([64, HID], F32, tag="woT_sb")
    ident_sb = persist.tile([128, 128], F32, tag="ident_sb")
    s_sb = persist.tile([128, 2], F32, tag="s_sb")
    t_sb = persist.tile([128, 2], F32, tag="t_sb")

    nc.gpsimd.dma_start(woT_sb[:], woT)
    nc.gpsimd.dma_start(ident_sb[:], ident)

    # ---------------- phase 1: projections + BN vectors ----------------
    with (
        tc.tile_pool(name="p1", bufs=1) as p1,
        tc.tile_pool(name="pp1", bufs=2, space=PSUM) as pp1,
        tc.tile_pool(name="ppv", bufs=2, space=PSUM) as ppv,
    ):
        k_sb = p1.tile([128, 2048], F32, tag="k_sb")
        q_sb = p1.tile([128, 2048], F32, tag="q_sb")
        v_sb = p1.tile([128, 2048], F32, tag="v_sb")
        nc.sync.dma_start(k_sb[:].rearrange("p (c n) -> p c n", c=2),
                          kb.rearrange("(c p) n -> p c n", p=128))
        nc.sync.dma_start(q_sb[:].rearrange("p (c n) -> p c n", c=2),
                          qb.rearrange("(c p) n -> p c n", p=128))
        nc.sync.dma_start(v_sb[:].rearrange("p (c n) -> p c n", c=2),
                          vb.rearrange("(c p) n -> p c n", p=128))
        wk_sb = p1.tile([128, 128], F32, tag="wk_sb")
        wv_sb = p1.tile([128, 128], F32, tag="wv_sb")
        wq_sb = p1.tile([128, 64], F32, tag="wq_sb")
        nc.gpsimd.dma_start(wk_sb[:].rearrange("p (c o) -> p c o", c=2),
                            wkT.rearrange("(c p) o -> p c o", p=128))
        nc.gpsimd.dma_start(wv_sb[:].rearrange("p (c o) -> p c o", c=2),
                            wvT.rearrange("(c p) o -> p c o", p=128))
        nc.gpsimd.dma_start(wq_sb[:].rearrange("p (c o) -> p c o", c=2),
                            wqT.rearrange("(c p) o -> p c o", p=128))

        # BN affine: s = gamma * rsqrt(var+eps), t = (bo - mean) * s + beta
        bn_sb = p1.tile([128, 10], F32, tag="bn_sb")
        nc.gpsimd.dma_start(bn_sb[:], bnv)
        tmp = p1.tile([128, 2], F32, tag="tmp")
        tmp2 = p1.tile([128, 2], F32, tag="tmp2")
        nc.vector.tensor_scalar_add(tmp[:], bn_sb[:, 6:8], BN_EPS)
        nc.scalar.sqrt(tmp[:], tmp[:])
        nc.vector.reciprocal(tmp[:], tmp[:])
        nc.vector.tensor_mul(s_sb[:], bn_sb[:, 0:2], tmp[:])
        nc.vector.tensor_sub(tmp2[:], bn_sb[:, 8:10], bn_sb[:, 4:6])
        nc.vector.tensor_mul(tmp2[:], tmp2[:], s_sb[:])
        nc.vector.tensor_add(t_sb[:], tmp2[:], bn_sb[:, 2:4])

        # K projection -> Kp2[d, 1024*j + n']
        for j in range(16):
            psj = pp1.tile([4, 1024], F32, tag="psj")
            for nn2 in range(2):
                for c in range(2):
                    nc.tensor.matmul(
                        psj[:, 512 * nn2:512 * nn2 + 512],
                        wk_sb[:, 64 * c + j:64 * c + j + 49:16],
                        k_sb[:, 1024 * c + 512 * nn2:1024 * c + 512 * nn2 + 512],
                        start=(c == 0), stop=(c == 1),
                    )
            nc.scalar.copy(Kp2[:, 1024 * j:1024 * j + 1024], psj[:])

        # Q projection (32 sub-channels, pre-scaled by SCALE)
        for j in range(8):
            psq = pp1.tile([4, 1024], F32, tag="psj")
            for nn2 in range(2):
                for c in range(2):
                    nc.tensor.matmul(
                        psq[:, 512 * nn2:512 * nn2 + 512],
                        wq_sb[:, 32 * c + 4 * j:32 * c + 4 * j + 4],
                        q_sb[:, 1024 * c + 512 * nn2:1024 * c + 512 * nn2 + 512],
                        start=(c == 0), stop=(c == 1),
                    )
            nc.scalar.mul(Qp2[:, 1024 * j:1024 * j + 1024], psq[:], SCALE)

        # V projection. Vt free layout per s-block of 80: cols [0,16) = ones
        # (denominator column, placed FIRST so it lands on the 32-aligned
        # psum row), cols [16+16d+j] = V projection.
        # Vt[r, 80*s + 16 + c] = sum_ch v[ch, 16r+s] * Wv[c, ch]
        for s in range(16):
            psv = ppv.tile([64, 64], F32, tag="psv")
            for c in range(2):
                nc.tensor.matmul(
                    psv[:],
                    v_sb[:, 1024 * c + s:1024 * c + s + 1009:16],
                    wv_sb[:, 64 * c:64 * c + 64],
                    start=(c == 0), stop=(c == 1),
                )
            nc.vector.tensor_copy(Vt[0:64, 80 * s + 16:80 * s + 80], psv[:])
        ones_view = Vt[0:64, :].rearrange("p (s c) -> p s c", c=80)[:, :, 0:16]
        nc.vector.memset(ones_view, 1.0)
        nc.gpsimd.dma_start(Vt[64:128, :], Vt[0:64, :])

    # ---------------- phase 2: attention ----------------
    with (
        tc.tile_pool(name="bias", bufs=2) as bp,
        tc.tile_pool(name="exp", bufs=16) as ep,
        tc.tile_pool(name="sml", bufs=2) as sp,
        tc.tile_pool(name="ps_s", bufs=3, space=PSUM) as pss,
        tc.tile_pool(name="ps_x", bufs=2, space=PSUM) as psx,
    ):
        Kv = Kp2[:].rearrange("d (m s) -> d m s", s=16)     # [4, 1024, 16]
        Qv = Qp2[:].rearrange("d (n s) -> d n s", s=16)     # [4, 512, 16]
        for g in range(4):                                   # head groups of 4
            # 4 heads' attn@V outputs col-tiled at psum rows {32k .. 32k+5}:
            # row 32k = softmax denominator (ones column), rows 32k+1..+5 = x.
            psa = psx.tile([128, NH], F32, tag="psa")
            psb = psx.tile([128, NH], F32, tag="psb")
            for k in range(4):
                h = 4 * g + k
                bh = bp.tile([128, 4096], F32, tag="bh")
                nc.sync.dma_start(bh[:].rearrange("p (t n) -> p t n", t=8),
                                  biasT[h].rearrange("(t p) n -> p t n", p=128))
                exps = []
                for t in range(8):
                    ps = pss.tile([128, NH], F32, tag="ps")
                    nc.tensor.matmul(ps[:], Kv[:, 128 * t:128 * t + 128, h], Qv[:, :, h],
                                     start=True, stop=False)
                    nc.tensor.matmul(ps[:], ident_sb[:], bh[:, 512 * t:512 * t + 512],
                                     start=False, stop=True)
                    ex = ep.tile([128, NH], F32, tag="ex")
                    nc.scalar.activation(ex[:], ps[:], AF.Exp)
                    exps.append(ex)
                for t in range(8):
                    nc.tensor.matmul(
                        psa[32 * k:32 * k + 5, :],
                        Vt[0:64, 80 * h + 2 * t:80 * h + 2 * t + 65:16],
                        exps[t][0:64, :], start=(t == 0), stop=(t == 7),
                        tile_position=(0, 32 * k))
                    nc.tensor.matmul(
                        psb[32 * k:32 * k + 5, :],
                        Vt[64:128, 80 * h + 2 * t + 1:80 * h + 2 * t + 66:16],
                        exps[t][64:128, :], start=(t == 0), stop=(t == 7),
                        tile_position=(64, 32 * k))
            for k in range(4):
                h = 4 * g + k
                d5 = sp.tile([5, NH], F32, tag="d5")
                nc.scalar.copy(d5[:], psa[32 * k:32 * k + 5, :])
                nc.vector.tensor_add(d5[:], d5[:], psb[32 * k:32 * k + 5, :])
                rec1 = sp.tile([1, NH], F32, tag="rec1")
                nc.vector.reciprocal(rec1[:], d5[0:1, :])
                r5 = sp.tile([5, NH], F32, tag="r5")
                nc.gpsimd.partition_broadcast(r5[:], rec1[:])
                m5 = sp.tile([5, NH], F32, tag="m5")
                nc.vector.tensor_mul(m5[:], d5[:], r5[:])
                nc.gpsimd.dma_start(x_sb[4 * h:4 * h + 4, :], m5[1:5, :])

        # ---------------- phase 3: output conv + BN + LeakyReLU ----------------
        for u in range(2):
            psy = pss.tile([128, NH], F32, tag="ps")
            nc.tensor.matmul(psy[:], woT_sb[0:64, 128 * u:128 * u + 128], x_sb[:],
                             start=True, stop=True)
            y2 = sp.tile([128, NH], F32, tag="y2")
            nc.vector.tensor_scalar(y2[:], psy[:], s_sb[:, u:u + 1], t_sb[:, u:u + 1],
                                    ALU.mult, ALU.add)
            yt = sp.tile([128, NH], F32, tag="yt")
            nc.vector.scalar_tensor_tensor(yt[:], y2[:], NEG_SLOPE, y2[:],
                                           ALU.mult, ALU.max)
            nc.sync.dma_start(y[128 * u:128 * u + 128, :], yt[:])


def build_program():
    nc = bacc.Bacc("TRN2", target_bir_lowering=False, debug=False)
    io = {
        "kb": nc.dram_tensor("kb", [HID, N], F32, kind="ExternalInput").ap(),
        "qb": nc.dram_tensor("qb", [HID, N], F32, kind="ExternalInput").ap(),
        "vb": nc.dram_tensor("vb", [HID, N], F32, kind="ExternalInput").ap(),
        "biasT": nc.dram_tensor("biasT", [H, N, NH], F32, kind="ExternalInput").ap(),
        "wkT": nc.dram_tensor("wkT", [HID, 64], F32, kind="ExternalInput").ap(),
        "wvT": nc.dram_tensor("wvT", [HID, 64], F32, kind="ExternalInput").ap(),
        "wqT": nc.dram_tensor("wqT", [HID, 32], F32, kind="ExternalInput").ap(),
        "woT": nc.dram_tensor("woT", [64, HID], F32, kind="ExternalInput").ap(),
        "bnv": nc.dram_tensor("bnv", [128, 10], F32, kind="ExternalInput").ap(),
        "ident": nc.dram_tensor("ident", [128, 128], F32, kind="ExternalInput").ap(),
        "y": nc.dram_tensor("y", [HID, NH], F32, kind="ExternalOutput").ap(),
    }
    with tile.TileContext(nc) as tc:
        _emit(nc, tc, io)
    nc.compile()
    return nc


def make_in_maps(q, k, v, attn_bias, Wq, Wk, Wv, Wo, bo, gamma, beta, run_mean, run_var):
    def f32(x):
        return np.ascontiguousarray(np.asarray(x, dtype=np.float32))

    q, k, v, attn_bias = f32(q), f32(k), f32(v), f32(attn_bias)
    Wq, Wk, Wv, Wo, bo = f32(Wq), f32(Wk), f32(Wv), f32(Wo), f32(bo)
    gamma, beta, run_mean, run_var = f32(gamma), f32(beta), f32(run_mean), f32(run_var)

    wkT = f32(Wk.T)
    wvT = f32(Wv.T)
    woT = f32(Wo.T)
    bnv = np.concatenate(
        [x.reshape(2, 128).T for x in (gamma, beta, run_mean, run_var, bo)], axis=1
    )
    bnv = f32(bnv)
    ident = np.eye(128, dtype=np.float32)

    in_maps = []
    for core in range(NCORES):
        b, half = divmod(core, 2)
        n0 = half * NH
        rows = np.array([16 * d + 8 * half + jl for jl in range(8) for d in range(4)])
        wqT = f32(Wq[rows, :].T)                                  # [256, 32], col = 4*jl+d
        biasT = f32(attn_bias[b, :, n0:n0 + NH, :].transpose(0, 2, 1))  # [16, 1024, 512]
        in_maps.append({
            "kb": f32(k[b]), "qb": f32(q[b]), "vb": f32(v[b]),
            "biasT": biasT, "wkT": wkT, "wvT": wvT, "wqT": wqT, "woT": woT,
            "bnv": bnv, "ident": ident,
        })
    return in_maps


_NC_CACHE = None


def get_nc():
    global _NC_CACHE
    if _NC_CACHE is None:
        _NC_CACHE = build_program()
    return _NC_CACHE


def kernel(**inputs):
    nc = get_nc()
    in_maps = make_in_maps(**inputs)
    res = run_bass_kernel_spmd(nc, in_maps, list(range(NCORES)))
    out = np.empty((B, HID, N), dtype=np.float32)
    for core in range(NCORES):
        b, half = divmod(core, 2)
        out[b, :, half * NH:(half + 1) * NH] = res.results[core]["y"]
    return out


# revision 15
# speedup vs baseline: 2.7322x; 2.7322x over previous
"""Trainium2 Bass kernel for nn_MultiHeadAttention_80418967650946.

Reference computation (per batch b):
  qp/kp/vp = 1x1-conv projections of q/k/v   [64, N]
  funky head view: qh[h,n,d] = qp.reshape(4, 16*N)[d, 16n+h]  (same for kh, vh)
  scores = qh @ kh * 0.25^0.5 + bias ; attn = softmax(scores)
  x[4h+d, n] = (attn @ vh)[h, n, d] ; y = LeakyReLU(BN(Wo @ x + bo), 0.2)

Sharding: 8 cores = 4 batches x 2 query-halves (n in [0,512) or [512,1024)).
Each core computes its query-half for ALL 16 heads fully locally (no
collectives): the output conv is column-wise independent, so y[:, n-half]
only needs x[:, n-half].

Per-core device algorithm (all fp32):
  - projections on TensorE produce Kp2 [4, 16384] (d-major, J=16m+h free),
    Qp2 [4, 8192] (pre-scaled by 0.25^0.5), and Vt [128, 1280] where
    Vt[r, 80s+c] = VpT[16r+s, c] for c<64 and 1.0 for c in [64,80)
    (rows 64:128 duplicate 0:64 for the second K=64 row-group).
  - scoresT[m-chunk, n] psum tiles come from one K=4 matmul; the additive
    bias is injected into the SAME psum by an identity matmul over the
    (host-pre-transposed) bias tile; exp() runs on ScalarE psum->sbuf.
  - attn@V contracts m on partitions via two K=64 row-tiled matmuls whose
    lhsT carries a ones column -> softmax denominators come for free.
  - per-head normalization assembles x into a PSUM tile (PSUM APs have no
    32-partition base alignment restriction), then one copy -> SBUF feeds
    the output matmul + fused BN-affine + LeakyReLU epilogue.
"""
import sys

if "/opt/trn_rl_repo" not in sys.path:
    sys.path.insert(0, "/opt/trn_rl_repo")

import numpy as np

import concourse.bass as bass
import concourse.tile as tile
from concourse import bacc, mybir
from concourse.bass_utils import run_bass_kernel_spmd

F32 = mybir.dt.float32
AF = mybir.ActivationFunctionType
ALU = mybir.AluOpType
PSUM = bass.MemorySpace.PSUM
F32R = mybir.dt.float32r




H = 16
D = 4
HID = 256
B = 4
N = 1024
NH = 512          # per-core query positions
NCORES = 8
SCALE = float(D) ** -0.5
BN_EPS = 1e-5
NEG_SLOPE = 0.2


def _emit(nc, tc, io):
    kb, qb, vb = io["kb"], io["qb"], io["vb"]
    biasT, wkT, wvT, wqT, woT = io["biasT"], io["wkT"], io["wvT"], io["wqT"], io["woT"]
    bnv, y = io["bnv"], io["y"]

    with tc.tile_pool(name="persist", bufs=1) as persist:
        _emit_body(nc, tc, io, persist)


def _emit_body(nc, tc, io, persist):
    kb, qb, vb = io["kb"], io["qb"], io["vb"]
    biasT, wkT, wvT, wqT, woT = io["biasT"], io["wkT"], io["wvT"], io["wqT"], io["woT"]
    bnv, y = io["bnv"], io["y"]

    Kp2 = persist.tile([36, H * N], F32R, tag="Kp2")
    Qp2 = persist.tile([36, H * NH], F32R, tag="Qp2")
    Vt = persist.tile([128, 16 * 80], F32R, tag="Vt")
    x_sb = persist.tile([64, NH], F32R, tag="x_sb")
    woT_sb = persist.tile([64, HID], F32R, tag="woT_sb")
    s_sb = persist.tile([128, 2], F32, tag="s_sb")
    t_sb = persist.tile([128, 2], F32, tag="t_sb")

    nc.gpsimd.dma_start(woT_sb[:], woT)

    # ---------------- phase 1: projections + BN vectors ----------------
    with (
        tc.tile_pool(name="p1", bufs=1) as p1,
        tc.tile_pool(name="pp1", bufs=2, space=PSUM) as pp1,
        tc.tile_pool(name="ppv", bufs=2, space=PSUM) as ppv,
    ):
        k_sb = p1.tile([128, 2048], F32R, tag="k_sb")
        q_sb = p1.tile([128, 2048], F32R, tag="q_sb")
        v_sb = p1.tile([128, 2048], F32, tag="v_sb")
        nc.gpsimd.dma_start(k_sb[:].rearrange("p (c n) -> p c n", c=2),
                            kb.rearrange("(c p) n -> p c n", p=128))
        nc.gpsimd.dma_start(q_sb[:].rearrange("p (c n) -> p c n", c=2),
                            qb.rearrange("(c p) n -> p c n", p=128))
        nc.sync.dma_start(v_sb[:].rearrange("p (c n) -> p c n", c=2),
                          vb.rearrange("(c p) n -> p c n", p=128))
        wk_sb = p1.tile([128, 128], F32R, tag="wk_sb")
        wv_sb = p1.tile([128, 128], F32, tag="wv_sb")
        wq_sb = p1.tile([128, 64], F32R, tag="wq_sb")
        nc.gpsimd.dma_start(wk_sb[:].rearrange("p (c o) -> p c o", c=2),
                            wkT.rearrange("(c p) o -> p c o", p=128))
        nc.gpsimd.dma_start(wv_sb[:].rearrange("p (c o) -> p c o", c=2),
                            wvT.rearrange("(c p) o -> p c o", p=128))
        nc.gpsimd.dma_start(wq_sb[:].rearrange("p (c o) -> p c o", c=2),
                            wqT.rearrange("(c p) o -> p c o", p=128))

        # BN affine: s = gamma * rsqrt(var+eps), t = (bo - mean) * s + beta
        bn_sb = p1.tile([128, 10], F32, tag="bn_sb")
        nc.gpsimd.dma_start(bn_sb[:], bnv)
        tmp = p1.tile([128, 2], F32, tag="tmp")
        tmp2 = p1.tile([128, 2], F32, tag="tmp2")
        nc.vector.tensor_scalar_add(tmp[:], bn_sb[:, 6:8], BN_EPS)
        nc.scalar.sqrt(tmp[:], tmp[:])
        nc.vector.reciprocal(tmp[:], tmp[:])
        nc.vector.tensor_mul(s_sb[:], bn_sb[:, 0:2], tmp[:])
        nc.vector.tensor_sub(tmp2[:], bn_sb[:, 8:10], bn_sb[:, 4:6])
        nc.vector.tensor_mul(tmp2[:], tmp2[:], s_sb[:])
        nc.vector.tensor_add(t_sb[:], tmp2[:], bn_sb[:, 2:4])

        # K projection -> Kp2[d, 1024*j + n']
        for j in range(16):
            psj = pp1.tile([4, 1024], F32, tag="psj")
            for nn2 in range(2):
                for c in range(2):
                    nc.tensor.matmul(
                        psj[:, 512 * nn2:512 * nn2 + 512],
                        wk_sb[:, 64 * c + j:64 * c + j + 49:16],
                        k_sb[:, 1024 * c + 512 * nn2:1024 * c + 512 * nn2 + 512],
                        start=(c == 0), stop=(c == 1),
                    )
            nc.scalar.copy(Kp2[0:4, 1024 * j:1024 * j + 1024], psj[:])

        # Q projection (32 sub-channels, pre-scaled by SCALE)
        for j in range(8):
            psq = pp1.tile([4, 1024], F32, tag="psj")
            for nn2 in range(2):
                for c in range(2):
                    nc.tensor.matmul(
                        psq[:, 512 * nn2:512 * nn2 + 512],
                        wq_sb[:, 32 * c + 4 * j:32 * c + 4 * j + 4],
                        q_sb[:, 1024 * c + 512 * nn2:1024 * c + 512 * nn2 + 512],
                        start=(c == 0), stop=(c == 1),
                    )
            nc.scalar.mul(Qp2[0:4, 1024 * j:1024 * j + 1024], psq[:], SCALE)

        # V projection. Vt free layout per s-block of 80: cols [0,16) = ones
        # (denominator column, placed FIRST so it lands on the 32-aligned
        # psum row), cols [16+16d+j] = V projection.
        # Vt[r, 80*s + 16 + c] = sum_ch v[ch, 16r+s] * Wv[c, ch]
        for s in range(16):
            psv = ppv.tile([64, 64], F32, tag="psv")
            for c in range(2):
                nc.tensor.matmul(
                    psv[:],
                    v_sb[:, 1024 * c + s:1024 * c + s + 1009:16],
                    wv_sb[:, 64 * c:64 * c + 64],
                    start=(c == 0), stop=(c == 1),
                )
            nc.vector.tensor_copy(Vt[0:64, 80 * s + 16:80 * s + 80], psv[:])
        ones_f32 = p1.tile([64, 256], F32, tag="ones_f32")
        nc.vector.memset(ones_f32[:], 1.0)
        ones_view = Vt[0:64, :].rearrange("p (s c) -> p s c", c=80)[:, :, 0:16]
        nc.vector.tensor_copy(ones_view, ones_f32[:].rearrange("p (s c) -> p s c", c=16))
        nc.gpsimd.dma_start(Vt[64:128, :], Vt[0:64, :])
        nc.gpsimd.dma_start(Kp2[32:36, :], Kp2[0:4, :])
        nc.gpsimd.dma_start(Qp2[32:36, :], Qp2[0:4, :])

    # ---------------- phase 2: attention ----------------
    with (
        tc.tile_pool(name="bias", bufs=2) as bp,
        tc.tile_pool(name="exp", bufs=16) as ep,
        tc.tile_pool(name="sml", bufs=2) as sp,
        tc.tile_pool(name="ps_s", bufs=4, space=PSUM) as pss,
        tc.tile_pool(name="ps_x", bufs=2, space=PSUM) as psx,
    ):
        Kv = [Kp2[0:4, :].rearrange("d (m s) -> d m s", s=16),
              Kp2[32:36, :].rearrange("d (m s) -> d m s", s=16)]   # [4, 1024, 16]
        Qv = [Qp2[0:4, :].rearrange("d (n s) -> d n s", s=16),
              Qp2[32:36, :].rearrange("d (n s) -> d n s", s=16)]   # [4, 512, 16]
        for h in range(H):
            bh = bp.tile([128, 4096], F32, tag="bh")
            nc.sync.dma_start(bh[:].rearrange("p (t n) -> p t n", t=8),
                              biasT[h].rearrange("(t p) n -> p t n", p=128))
            exps = []
            for t in range(8):
                rg = t % 2     # alternate PE row groups (0,0)/(32,0)
                ps = pss.tile([128, NH], F32, tag="ps")
                nc.tensor.matmul(ps[:], Kv[rg][:, 128 * t:128 * t + 128, h],
                                 Qv[rg][:, :, h],
                                 start=True, stop=True, tile_position=(32 * rg, 0))
                nc.vector.tensor_add(ps[:], ps[:], bh[:, 512 * t:512 * t + 512])
                ex = ep.tile([128, NH], F32R, tag="ex")
                nc.scalar.activation(ex[:], ps[:], AF.Exp)
                exps.append(ex)
            # attn@V: two K=64 row-groups; lhsT column 0 is the ones column,
            # so psum row 0 = softmax denominator, rows 1..5 = x (unnormalized)
            psa = psx.tile([5, NH], F32, tag="psa")
            psb = psx.tile([5, NH], F32, tag="psb")
            for t in range(8):
                nc.tensor.matmul(
                    psa[:],
                    Vt[0:64, 80 * h + 2 * t:80 * h + 2 * t + 65:16],
                    exps[t][0:64, :], start=(t == 0), stop=(t == 7),
                    tile_position=(0, 0))
                nc.tensor.matmul(
                    psb[:],
                    Vt[64:128, 80 * h + 2 * t + 1:80 * h + 2 * t + 66:16],
                    exps[t][64:128, :], start=(t == 0), stop=(t == 7),
                    tile_position=(64, 0))
            d5 = sp.tile([5, NH], F32, tag="d5")
            nc.scalar.copy(d5[:], psa[:])
            nc.vector.tensor_add(d5[:], d5[:], psb[:])
            rec1 = sp.tile([1, NH], F32, tag="rec1")
            nc.vector.reciprocal(rec1[:], d5[0:1, :])
            r5 = sp.tile([5, NH], F32, tag="r5")
            nc.gpsimd.partition_broadcast(r5[:], rec1[:])
            m5 = sp.tile([5, NH], F32R, tag="m5")
            nc.vector.tensor_mul(m5[:], d5[:], r5[:])
            nc.gpsimd.dma_start(x_sb[4 * h:4 * h + 4, :], m5[1:5, :])

        # ---------------- phase 3: output conv + BN + LeakyReLU ----------------
        for u in range(2):
            psy = pss.tile([128, NH], F32, tag="ps")
            nc.tensor.matmul(psy[:], woT_sb[0:64, 128 * u:128 * u + 128], x_sb[:],
                             start=True, stop=True)
            y2 = sp.tile([128, NH], F32, tag="y2")
            nc.vector.tensor_scalar(y2[:], psy[:], s_sb[:, u:u + 1], t_sb[:, u:u + 1],
                                    ALU.mult, ALU.add)
            yt = sp.tile([128, NH], F32, tag="yt")
            nc.vector.scalar_tensor_tensor(yt[:], y2[:], NEG_SLOPE, y2[:],
                                           ALU.mult, ALU.max)
            nc.sync.dma_start(y[128 * u:128 * u + 128, :], yt[:])


def build_program():
    nc = bacc.Bacc("TRN2", target_bir_lowering=False, debug=False)
    io = {
        "kb": nc.dram_tensor("kb", [HID, N], F32, kind="ExternalInput").ap(),
        "qb": nc.dram_tensor("qb", [HID, N], F32, kind="ExternalInput").ap(),
        "vb": nc.dram_tensor("vb", [HID, N], F32, kind="ExternalInput").ap(),
        "biasT": nc.dram_tensor("biasT", [H, N, NH], F32, kind="ExternalInput").ap(),
        "wkT": nc.dram_tensor("wkT", [HID, 64], F32, kind="ExternalInput").ap(),
        "wvT": nc.dram_tensor("wvT", [HID, 64], F32, kind="ExternalInput").ap(),
        "wqT": nc.dram_tensor("wqT", [HID, 32], F32, kind="ExternalInput").ap(),
        "woT": nc.dram_tensor("woT", [64, HID], F32, kind="ExternalInput").ap(),
        "bnv": nc.dram_tensor("bnv", [128, 10], F32, kind="ExternalInput").ap(),
        "y": nc.dram_tensor("y", [HID, NH], F32, kind="ExternalOutput").ap(),
    }
    with tile.TileContext(nc) as tc:
        _emit(nc, tc, io)
    nc.compile()
    return nc


def make_in_maps(q, k, v, attn_bias, Wq, Wk, Wv, Wo, bo, gamma, beta, run_mean, run_var):
    def f32(x):
        return np.ascontiguousarray(np.asarray(x, dtype=np.float32))

    q, k, v, attn_bias = f32(q), f32(k), f32(v), f32(attn_bias)
    Wq, Wk, Wv, Wo, bo = f32(Wq), f32(Wk), f32(Wv), f32(Wo), f32(bo)
    gamma, beta, run_mean, run_var = f32(gamma), f32(beta), f32(run_mean), f32(run_var)

    wkT = f32(Wk.T)
    wvT = f32(Wv.T)
    woT = f32(Wo.T)
    bnv = np.concatenate(
        [x.reshape(2, 128).T for x in (gamma, beta, run_mean, run_var, bo)], axis=1
    )
    bnv = f32(bnv)

    in_maps = []
    for core in range(NCORES):
        b, half = divmod(core, 2)
        n0 = half * NH
        rows = np.array([16 * d + 8 * half + jl for jl in range(8) for d in range(4)])
        wqT = f32(Wq[rows, :].T)                                  # [256, 32], col = 4*jl+d
        biasT = f32(attn_bias[b, :, n0:n0 + NH, :].transpose(0, 2, 1))  # [16, 1024, 512]
        in_maps.append({
            "kb": f32(k[b]), "qb": f32(q[b]), "vb": f32(v[b]),
            "biasT": biasT, "wkT": wkT, "wvT": wvT, "wqT": wqT, "woT": woT,
            "bnv": bnv,
        })
    return in_maps


_NC_CACHE = None


def get_nc():
    global _NC_CACHE
    if _NC_CACHE is None:
        _NC_CACHE = build_program()
    return _NC_CACHE


def kernel(**inputs):
    nc = get_nc()
    in_maps = make_in_maps(**inputs)
    res = run_bass_kernel_spmd(nc, in_maps, list(range(NCORES)))
    out = np.empty((B, HID, N), dtype=np.float32)
    for core in range(NCORES):
        b, half = divmod(core, 2)
        out[b, :, half * NH:(half + 1) * NH] = res.results[core]["y"]
    return out


# revision 16
# speedup vs baseline: 2.9723x; 1.0879x over previous
"""Trainium2 Bass kernel for nn_MultiHeadAttention_80418967650946.

Reference computation (per batch b):
  qp/kp/vp = 1x1-conv projections of q/k/v   [64, N]
  funky head view: qh[h,n,d] = qp.reshape(4, 16*N)[d, 16n+h]  (same for kh, vh)
  scores = qh @ kh * 0.25^0.5 + bias ; attn = softmax(scores)
  x[4h+d, n] = (attn @ vh)[h, n, d] ; y = LeakyReLU(BN(Wo @ x + bo), 0.2)

Sharding: 8 cores = 4 batches x 2 query-halves (n in [0,512) or [512,1024)).
Each core computes its query-half for ALL 16 heads fully locally (no
collectives): the output conv is column-wise independent, so y[:, n-half]
only needs x[:, n-half].

Per-core device algorithm (all fp32):
  - projections on TensorE produce Kp2 [4, 16384] (d-major, J=16m+h free),
    Qp2 [4, 8192] (pre-scaled by 0.25^0.5), and Vt [128, 1280] where
    Vt[r, 80s+c] = VpT[16r+s, c] for c<64 and 1.0 for c in [64,80)
    (rows 64:128 duplicate 0:64 for the second K=64 row-group).
  - scoresT[m-chunk, n] psum tiles come from one K=4 matmul; the additive
    bias is injected into the SAME psum by an identity matmul over the
    (host-pre-transposed) bias tile; exp() runs on ScalarE psum->sbuf.
  - attn@V contracts m on partitions via two K=64 row-tiled matmuls whose
    lhsT carries a ones column -> softmax denominators come for free.
  - per-head normalization assembles x into a PSUM tile (PSUM APs have no
    32-partition base alignment restriction), then one copy -> SBUF feeds
    the output matmul + fused BN-affine + LeakyReLU epilogue.
"""
import sys

if "/opt/trn_rl_repo" not in sys.path:
    sys.path.insert(0, "/opt/trn_rl_repo")

import numpy as np

import concourse.bass as bass
import concourse.tile as tile
from concourse import bacc, mybir
from concourse.bass_utils import run_bass_kernel_spmd

F32 = mybir.dt.float32
AF = mybir.ActivationFunctionType
ALU = mybir.AluOpType
PSUM = bass.MemorySpace.PSUM
F32R = mybir.dt.float32r
BF16 = mybir.dt.bfloat16




H = 16
D = 4
HID = 256
B = 4
N = 1024
NH = 512          # per-core query positions
NCORES = 8
SCALE = float(D) ** -0.5
BN_EPS = 1e-5
NEG_SLOPE = 0.2


def _emit(nc, tc, io):
    kb, qb, vb = io["kb"], io["qb"], io["vb"]
    biasT, wkT, wvT, wqT, woT = io["biasT"], io["wkT"], io["wvT"], io["wqT"], io["woT"]
    bnv, y = io["bnv"], io["y"]

    with tc.tile_pool(name="persist", bufs=1) as persist:
        _emit_body(nc, tc, io, persist)


def _emit_body(nc, tc, io, persist):
    kb, qb, vb = io["kb"], io["qb"], io["vb"]
    biasT, wkT, wvT, wqT, woT = io["biasT"], io["wkT"], io["wvT"], io["wqT"], io["woT"]
    bnv, y = io["bnv"], io["y"]

    Kp2 = persist.tile([36, H * N], BF16, tag="Kp2")
    Qp2 = persist.tile([36, H * NH], BF16, tag="Qp2")
    Vt = persist.tile([128, 16 * 80], BF16, tag="Vt")
    x_sb = persist.tile([64, NH], F32R, tag="x_sb")
    woT_sb = persist.tile([64, HID], F32R, tag="woT_sb")
    s_sb = persist.tile([128, 2], F32, tag="s_sb")
    t_sb = persist.tile([128, 2], F32, tag="t_sb")

    nc.gpsimd.dma_start(woT_sb[:], woT)

    # ---------------- phase 1: projections + BN vectors ----------------
    with (
        tc.tile_pool(name="p1", bufs=1) as p1,
        tc.tile_pool(name="pp1", bufs=2, space=PSUM) as pp1,
        tc.tile_pool(name="ppv", bufs=2, space=PSUM) as ppv,
    ):
        k_sb = p1.tile([128, 2048], BF16, tag="k_sb")
        q_sb = p1.tile([128, 2048], BF16, tag="q_sb")
        v_sb = p1.tile([128, 2048], BF16, tag="v_sb")
        nc.gpsimd.dma_start(k_sb[:].rearrange("p (c n) -> p c n", c=2),
                            kb.rearrange("(c p) n -> p c n", p=128))
        nc.gpsimd.dma_start(q_sb[:].rearrange("p (c n) -> p c n", c=2),
                            qb.rearrange("(c p) n -> p c n", p=128))
        nc.gpsimd.dma_start(v_sb[:].rearrange("p (c n) -> p c n", c=2),
                            vb.rearrange("(c p) n -> p c n", p=128))
        wk_sb = p1.tile([128, 128], BF16, tag="wk_sb")
        wv_sb = p1.tile([128, 128], BF16, tag="wv_sb")
        wq_sb = p1.tile([128, 64], BF16, tag="wq_sb")
        nc.gpsimd.dma_start(wk_sb[:].rearrange("p (c o) -> p c o", c=2),
                            wkT.rearrange("(c p) o -> p c o", p=128))
        nc.gpsimd.dma_start(wv_sb[:].rearrange("p (c o) -> p c o", c=2),
                            wvT.rearrange("(c p) o -> p c o", p=128))
        nc.gpsimd.dma_start(wq_sb[:].rearrange("p (c o) -> p c o", c=2),
                            wqT.rearrange("(c p) o -> p c o", p=128))

        # BN affine: s = gamma * rsqrt(var+eps), t = (bo - mean) * s + beta
        bn_sb = p1.tile([128, 10], F32, tag="bn_sb")
        nc.gpsimd.dma_start(bn_sb[:], bnv)
        tmp = p1.tile([128, 2], F32, tag="tmp")
        tmp2 = p1.tile([128, 2], F32, tag="tmp2")
        nc.vector.tensor_scalar_add(tmp[:], bn_sb[:, 6:8], BN_EPS)
        nc.scalar.sqrt(tmp[:], tmp[:])
        nc.vector.reciprocal(tmp[:], tmp[:])
        nc.vector.tensor_mul(s_sb[:], bn_sb[:, 0:2], tmp[:])
        nc.vector.tensor_sub(tmp2[:], bn_sb[:, 8:10], bn_sb[:, 4:6])
        nc.vector.tensor_mul(tmp2[:], tmp2[:], s_sb[:])
        nc.vector.tensor_add(t_sb[:], tmp2[:], bn_sb[:, 2:4])

        # K projection -> Kp2[d, 1024*j + n']
        for j in range(16):
            psj = pp1.tile([4, 1024], F32, tag="psj")
            for nn2 in range(2):
                for c in range(2):
                    nc.tensor.matmul(
                        psj[:, 512 * nn2:512 * nn2 + 512],
                        wk_sb[:, 64 * c + j:64 * c + j + 49:16],
                        k_sb[:, 1024 * c + 512 * nn2:1024 * c + 512 * nn2 + 512],
                        start=(c == 0), stop=(c == 1),
                    )
            nc.scalar.copy(Kp2[0:4, 1024 * j:1024 * j + 1024], psj[:])

        # Q projection (32 sub-channels, pre-scaled by SCALE)
        for j in range(8):
            psq = pp1.tile([4, 1024], F32, tag="psj")
            for nn2 in range(2):
                for c in range(2):
                    nc.tensor.matmul(
                        psq[:, 512 * nn2:512 * nn2 + 512],
                        wq_sb[:, 32 * c + 4 * j:32 * c + 4 * j + 4],
                        q_sb[:, 1024 * c + 512 * nn2:1024 * c + 512 * nn2 + 512],
                        start=(c == 0), stop=(c == 1),
                    )
            nc.scalar.mul(Qp2[0:4, 1024 * j:1024 * j + 1024], psq[:], SCALE)

        # V projection. Vt free layout per s-block of 80: cols [0,16) = ones
        # (denominator column, placed FIRST so it lands on the 32-aligned
        # psum row), cols [16+16d+j] = V projection.
        # Vt[r, 80*s + 16 + c] = sum_ch v[ch, 16r+s] * Wv[c, ch]
        for s in range(16):
            psv = ppv.tile([64, 64], F32, tag="psv")
            for c in range(2):
                nc.tensor.matmul(
                    psv[:],
                    v_sb[:, 1024 * c + s:1024 * c + s + 1009:16],
                    wv_sb[:, 64 * c:64 * c + 64],
                    start=(c == 0), stop=(c == 1),
                )
            nc.vector.tensor_copy(Vt[0:64, 80 * s + 16:80 * s + 80], psv[:])
        ones_f32 = p1.tile([64, 256], F32, tag="ones_f32")
        nc.vector.memset(ones_f32[:], 1.0)
        ones_view = Vt[0:64, :].rearrange("p (s c) -> p s c", c=80)[:, :, 0:16]
        nc.vector.tensor_copy(ones_view, ones_f32[:].rearrange("p (s c) -> p s c", c=16))
        nc.gpsimd.dma_start(Vt[64:128, :], Vt[0:64, :])
        nc.gpsimd.dma_start(Kp2[32:36, :], Kp2[0:4, :])
        nc.gpsimd.dma_start(Qp2[32:36, :], Qp2[0:4, :])

    # ---------------- phase 2: attention ----------------
    with (
        tc.tile_pool(name="bias", bufs=2) as bp,
        tc.tile_pool(name="exp", bufs=16) as ep,
        tc.tile_pool(name="sml", bufs=2) as sp,
        tc.tile_pool(name="ps_s", bufs=2, space=PSUM) as pss,
        tc.tile_pool(name="ps_x", bufs=2, space=PSUM) as psx,
    ):
        Kv = [Kp2[0:4, :].rearrange("d (m s) -> d m s", s=16),
              Kp2[32:36, :].rearrange("d (m s) -> d m s", s=16)]   # [4, 1024, 16]
        Qv = [Qp2[0:4, :].rearrange("d (n s) -> d n s", s=16),
              Qp2[32:36, :].rearrange("d (n s) -> d n s", s=16)]   # [4, 512, 16]
        for h in range(H):
            if h % 2 == 0:      # one DMA loads the bias for two heads (4 MB)
                bh2 = bp.tile([128, 8192], F32, tag="bh2")
                nc.sync.dma_start(
                    bh2[:].rearrange("p (h t n) -> p h t n", h=2, t=8),
                    biasT[h:h + 2].rearrange("h (t p) n -> p h t n", p=128))
            hb = 4096 * (h % 2)
            exps = []
            for u in range(4):   # pairs of m-chunks -> one 2-bank psum tile
                ps = pss.tile([128, 1024], F32, tag="ps")
                for v2 in range(2):
                    t = 2 * u + v2
                    nc.tensor.matmul(ps[:, 512 * v2:512 * v2 + 512],
                                     Kv[v2][:, 128 * t:128 * t + 128, h],
                                     Qv[v2][:, :, h],
                                     start=True, stop=True,
                                     tile_position=(32 * v2, 0))
                nc.vector.tensor_add(ps[:], ps[:], bh2[:, hb + 1024 * u:hb + 1024 * u + 1024])
                ex = ep.tile([128, 1024], BF16, tag="ex")
                nc.scalar.activation(ex[:], ps[:], AF.Exp)
                exps.append(ex)
            # attn@V: two K=64 row-groups; lhsT column 0 is the ones column,
            # so psum row 0 = softmax denominator, rows 1..5 = x (unnormalized)
            psa = psx.tile([5, NH], F32, tag="psa")
            psb = psx.tile([5, NH], F32, tag="psb")
            for t in range(8):
                ex = exps[t // 2]
                eo = 512 * (t % 2)
                nc.tensor.matmul(
                    psa[:],
                    Vt[0:64, 80 * h + 2 * t:80 * h + 2 * t + 65:16],
                    ex[0:64, eo:eo + 512], start=(t == 0), stop=(t == 7),
                    tile_position=(0, 0))
                nc.tensor.matmul(
                    psb[:],
                    Vt[64:128, 80 * h + 2 * t + 1:80 * h + 2 * t + 66:16],
                    ex[64:128, eo:eo + 512], start=(t == 0), stop=(t == 7),
                    tile_position=(64, 0))
            d5 = sp.tile([5, NH], F32, tag="d5")
            nc.scalar.copy(d5[:], psa[:])
            nc.vector.tensor_add(d5[:], d5[:], psb[:])
            r5p = sp.tile([5, NH], F32, tag="r5p")
            nc.gpsimd.partition_broadcast(r5p[:], d5[0:1, :])
            r5 = sp.tile([5, NH], F32, tag="r5")
            nc.vector.reciprocal(r5[:], r5p[:])
            m5 = sp.tile([5, NH], F32R, tag="m5")
            nc.vector.tensor_mul(m5[:], d5[:], r5[:])
            nc.gpsimd.dma_start(x_sb[4 * h:4 * h + 4, :], m5[1:5, :])

        # ---------------- phase 3: output conv + BN + LeakyReLU ----------------
        for u in range(2):
            psy = pss.tile([128, NH], F32, tag="ps")
            nc.tensor.matmul(psy[:], woT_sb[0:64, 128 * u:128 * u + 128], x_sb[:],
                             start=True, stop=True)
            y2 = sp.tile([128, NH], F32, tag="y2")
            nc.vector.tensor_scalar(y2[:], psy[:], s_sb[:, u:u + 1], t_sb[:, u:u + 1],
                                    ALU.mult, ALU.add)
            yt = sp.tile([128, NH], F32, tag="yt")
            nc.vector.scalar_tensor_tensor(yt[:], y2[:], NEG_SLOPE, y2[:],
                                           ALU.mult, ALU.max)
            nc.sync.dma_start(y[128 * u:128 * u + 128, :], yt[:])


def build_program():
    nc = bacc.Bacc("TRN2", target_bir_lowering=False, debug=False)
    io = {
        "kb": nc.dram_tensor("kb", [HID, N], F32, kind="ExternalInput").ap(),
        "qb": nc.dram_tensor("qb", [HID, N], F32, kind="ExternalInput").ap(),
        "vb": nc.dram_tensor("vb", [HID, N], F32, kind="ExternalInput").ap(),
        "biasT": nc.dram_tensor("biasT", [H, N, NH], F32, kind="ExternalInput").ap(),
        "wkT": nc.dram_tensor("wkT", [HID, 64], F32, kind="ExternalInput").ap(),
        "wvT": nc.dram_tensor("wvT", [HID, 64], F32, kind="ExternalInput").ap(),
        "wqT": nc.dram_tensor("wqT", [HID, 32], F32, kind="ExternalInput").ap(),
        "woT": nc.dram_tensor("woT", [64, HID], F32, kind="ExternalInput").ap(),
        "bnv": nc.dram_tensor("bnv", [128, 10], F32, kind="ExternalInput").ap(),
        "y": nc.dram_tensor("y", [HID, NH], F32, kind="ExternalOutput").ap(),
    }
    with tile.TileContext(nc) as tc:
        _emit(nc, tc, io)
    nc.compile()
    return nc


def make_in_maps(q, k, v, attn_bias, Wq, Wk, Wv, Wo, bo, gamma, beta, run_mean, run_var):
    def f32(x):
        return np.ascontiguousarray(np.asarray(x, dtype=np.float32))

    q, k, v, attn_bias = f32(q), f32(k), f32(v), f32(attn_bias)
    Wq, Wk, Wv, Wo, bo = f32(Wq), f32(Wk), f32(Wv), f32(Wo), f32(bo)
    gamma, beta, run_mean, run_var = f32(gamma), f32(beta), f32(run_mean), f32(run_var)

    wkT = f32(Wk.T)
    wvT = f32(Wv.T)
    woT = f32(Wo.T)
    bnv = np.concatenate(
        [x.reshape(2, 128).T for x in (gamma, beta, run_mean, run_var, bo)], axis=1
    )
    bnv = f32(bnv)

    in_maps = []
    for core in range(NCORES):
        b, half = divmod(core, 2)
        n0 = half * NH
        rows = np.array([16 * d + 8 * half + jl for jl in range(8) for d in range(4)])
        wqT = f32(Wq[rows, :].T)                                  # [256, 32], col = 4*jl+d
        biasT = f32(attn_bias[b, :, n0:n0 + NH, :].transpose(0, 2, 1))  # [16, 1024, 512]
        in_maps.append({
            "kb": f32(k[b]), "qb": f32(q[b]), "vb": f32(v[b]),
            "biasT": biasT, "wkT": wkT, "wvT": wvT, "wqT": wqT, "woT": woT,
            "bnv": bnv,
        })
    return in_maps


_NC_CACHE = None


def get_nc():
    global _NC_CACHE
    if _NC_CACHE is None:
        _NC_CACHE = build_program()
    return _NC_CACHE


def kernel(**inputs):
    nc = get_nc()
    in_maps = make_in_maps(**inputs)
    res = run_bass_kernel_spmd(nc, in_maps, list(range(NCORES)))
    out = np.empty((B, HID, N), dtype=np.float32)
    for core in range(NCORES):
        b, half = divmod(core, 2)
        out[b, :, half * NH:(half + 1) * NH] = res.results[core]["y"]
    return out
